# revision 40
# baseline (speedup 1.0000x reference)
"""DFL loss (nn_DFLLoss) Trainium2 Bass kernel — 8-core data parallel.

reference computes, per (batch, pixel, coord j in 0..3):
    rl[b, hw, j, k] = reg_logits[b, j*8+k, hw]          (k in 0..7 bins)
    t = clip(targets, 0, 6.9999); l = floor(t); u = l+1
    per = w_l * (lse - rl[l]) + w_u * (lse - rl[u]),  lse = logsumexp_k rl
    loss = sum(per * pos_mask) / (max(sum(pos_mask), 1) * 4)

Key identity (removes the gather):
    w_l*rl[l] + w_u*rl[u] = sum_k relu(1 - |t - k|) * rl[k]
so masked_total = sum(mask*lse) - sum_k relu(1-|t-k|)*rl[k]*mask. The
hat-product+reduce runs as ONE fused custom DVE op per (batch, coord)
with the bin index k supplied by PageIdx over the 8 channel pages.
The mask is folded into t'' = t + 100*mask: the op evaluates
relu(1 - |t'' - (100 + k)|), so positive pixels give |t - k| and
masked-out pixels give |t - 100 - k| >= 93 -> all hat weights 0.
(Plain tent weights are exact linear-interp weights for t in [0,7);
the reference's clip to 6.9999 only perturbs a ~1e-5 pixel fraction
by <=1e-4.)

Performance model (TimelineSim): all DMA transfers serialize on one
DMA_ENGINES device at 360B/ns -> the 14.85MB/core input stream is a
~41.6us roofline. Engines issue IN ORDER per queue, so each engine's
program order must match data-arrival order or ready work gets stuck
behind stalled ops. Per-(b,j) quantum (2594ns of DMA):
  DVE : hat (custom ISA 1727) + s8 (bf16 278) + s4 (bf16 165) + ttr/4
  Act : exp f32->bf16 (1518) + ln/4 + [mf100 + npos Copy-accum]/4
  Pool: s16 (bf16 tt 1777) + t2 prep/4
Batch 0 preps on DVE (idle during ramp); batch 3 runs ln/ttr per-coord
and the last L tile is DMA'd in two halves (two hat calls, s0=100/104)
to shorten the post-stream tail.
"""

import threading
from operator import add as _operator_add

import numpy as np

BINS = 8
B, C, H, W = 32, 32, 160, 160
HW = H * W  # 25600
NCORES = 8
BPC = B // NCORES  # 4 batches per core
PX = HW // 128  # 200 pixels per partition per batch
NJ = 4

_lock = threading.Lock()
_cache: dict = {}


def _register_hat_op():
    """Register the fused hat*logit+reduce custom DVE op (idempotent)."""
    import concourse.dve_ops as dve_ops
    from concourse.dve_spec import (
        C0,
        C1,
        PageIdx,
        Spec,
        Src0,
        Src1,
        Zero,
        One,
        lower,
        maxx,
        relu,
    )
    from concourse.dve_uop import DveOpSpec

    name = "HAT_MUL_ACC_DFL"
    if name in dve_ops._SUB_OPCODE_FOR_NAME:
        for op in dve_ops.OPS:
            if op.name == name:
                return op

    _pg = PageIdx(C0, C1)  # idx = s0 + s1*page  (page = bin k)
    _d = Src0 - _pg

    def _ref(in0, in1, s0, s1, imm2):
        P, S, N = in0.shape
        idx = (s0 + s1 * np.arange(S)).reshape(1, S, 1)
        hat = np.maximum(1.0 - np.abs(in0.astype(np.float32) - idx), 0.0)
        body = (hat * in1).astype(np.float32)
        return body, body.reshape(P, -1).sum(-1, keepdims=True)

    spec = Spec(
        body=relu(One - maxx(_d, Zero - _d)) * Src1,
        accum=_operator_add,
        accum_init=Zero,
        reference=_ref,
    )
    shas = {}
    for ver in ("v3", "v4"):
        uops = lower(spec, ver=ver)
        shas[ver] = DveOpSpec(name=name, opcode=1, uops=uops, rd1_en=True).sha(ver)
    op = dve_ops.DveOp(name, spec, subdim=True, uops_sha=shas)
    row = dve_ops._CUSTOM_DVE_ROW_BASE + len(dve_ops.OPS)
    assert row < 0x20, "custom DVE opcode rows exhausted"
    dve_ops.OPS.append(op)
    dve_ops.CUSTOM_DVE_SPECS[name] = op.spec
    dve_ops._SUB_OPCODE_FOR_NAME[name] = row
    return op


def _patch_act_tables():
    """Force Exp/Ln/Copy/Identity to resolve to the one table set with all.

    The act-table-load pass assigns each activation the first set containing
    its function; without this, Copy (mf100/npos) would resolve to
    exp_and_others while Ln needs natural_log*, alternating ~1.3us table
    loads. Removing the four functions from every other set (list order and
    ids preserved) makes natural_log_exp_and_others serve all of them: one
    load for the whole kernel.
    """
    import concourse.bacc as bacc
    import concourse.hw_specs as hw_specs
    import concourse.mybir as mybir

    if getattr(_patch_act_tables, "_done", False):
        return
    orig = hw_specs.get_activation_tables
    A = mybir.ActivationFunctionType
    fns_pin = (A.Exp, A.Ln, A.Copy, A.Identity)

    def patched(module_arch):
        t = orig(module_arch)
        both = t.get("natural_log_exp_and_others")
        if both is not None and all(f in both for f in fns_pin):
            for name, fns in t.items():
                if name != "natural_log_exp_and_others":
                    for f in fns_pin:
                        fns.discard(f)
        return t

    hw_specs.get_activation_tables = patched
    bacc.get_activation_tables = patched
    _patch_act_tables._done = True


def _build_nc():
    import concourse.bacc as bacc
    import concourse.mybir as mybir
    from concourse.tile import TileContext
    from concourse.dve_ops import TENSOR_TENSOR_REDUCE as ttr_op

    _patch_act_tables()
    hat_op = _register_hat_op()
    f32 = mybir.dt.float32
    bf16 = mybir.dt.bfloat16
    u8 = mybir.dt.uint8

    nc = bacc.Bacc("TRN2", target_bir_lowering=False, debug=False)
    x = nc.dram_tensor("x", [BPC, C, HW], f32, kind="ExternalInput")
    tg = nc.dram_tensor("tg", [BPC, HW, NJ], f32, kind="ExternalInput")
    mk = nc.dram_tensor("mk", [BPC, HW], u8, kind="ExternalInput")
    # acc cols: 0..15 hat_u (u=15 -> first-half hat); 16..31 lse_u;
    # 32 hat15 second half; 33..36 100*npos per batch
    acc_out = nc.dram_tensor("acc", [128, 37], f32, kind="ExternalOutput")

    x_v = x.rearrange("b c (blk px) -> b blk c px", px=PX)  # [4,128,32,200]
    tg_v = tg.rearrange("b (blk pj) j -> b blk (pj j)", blk=128)  # [4,128,800]
    mk_v = mk.rearrange("b (blk px) -> b blk px", px=PX)  # [4,128,200]

    Exp = mybir.ActivationFunctionType.Exp
    Ln = mybir.ActivationFunctionType.Ln
    Copy = mybir.ActivationFunctionType.Copy
    Alu = mybir.AluOpType

    # ---- DMA stream time model (drives scheduler order via wait_until) ----
    T_DMA0, D_T, D_TC, D_M, D_L = 1966, 1138, 284, 142, 2276
    SEM = 950  # DMA sem-prop + receive latency
    t_end = [0.0] * BPC
    m_end = [0.0] * BPC
    L_end = [0.0] * (BPC * NJ + 1)  # [15]=first half, [16]=second half
    cur = T_DMA0
    # stream: t0 m0 L0 L1 L2 [t1 m1] L3 | L4 L5 L6 [t2 m2] L7 | ...
    for b in range(BPC):
        if b == 0:
            cur += D_T; t_end[0] = cur
            cur += D_M; m_end[0] = cur
        for j in range(NJ):
            u = b * NJ + j
            if u < BPC * NJ - 1:
                cur += D_L; L_end[u] = cur
            else:
                cur += D_L * 3 // 4; L_end[15] = cur
                cur += D_L // 4; L_end[16] = cur
            if j == 2 and b < BPC - 1:
                cur += D_T; t_end[b + 1] = cur
                cur += D_M; m_end[b + 1] = cur

    with TileContext(nc) as tc:
        from contextlib import contextmanager

        @contextmanager
        def at(ts_ns):
            with tc.tile_wait_until(ts_ns / 1e6):
                yield

        with (
            tc.tile_pool(name="pL", bufs=6) as pL,
            tc.tile_pool(name="pE", bufs=4) as pE,
            tc.tile_pool(name="pS", bufs=8) as pS,
            tc.tile_pool(name="pT", bufs=2) as pT,
            tc.tile_pool(name="pM", bufs=4) as pM,
            tc.tile_pool(name="pOnce", bufs=1) as pOnce,
        ):
            accs = pOnce.tile([128, 37], f32)

            t_raws = [None] * BPC
            m_raws = [None] * BPC
            mf100s = [None] * BPC
            t2s = [None] * BPC
            s16s = [None] * (BPC * NJ)
            s8s = [None] * (BPC * NJ)
            s4s = [None] * (BPC * NJ)
            lses = [None] * (BPC * NJ)

            def dma_tm(b):
                t_raws[b] = pT.tile([128, PX * NJ], f32, tag="t_raw", name="t_raw", bufs=2)
                m_raws[b] = pT.tile([128, PX], u8, tag="m_raw", name="m_raw", bufs=2)
                nc.sync.dma_start(t_raws[b][:, :], tg_v[b])
                nc.sync.dma_start(m_raws[b][:, :], mk_v[b])

            def dma_t_chunk(b, j):
                if j == 0:
                    t_raws[b] = pT.tile([128, PX * NJ], f32, tag="t_raw", name="t_raw", bufs=2)
                c0, c1 = j * PX, (j + 1) * PX
                nc.sync.dma_start(t_raws[b][:, c0:c1], tg_v[b][:, c0:c1])

            def dma_m(b):
                m_raws[b] = pT.tile([128, PX], u8, tag="m_raw", name="m_raw", bufs=2)
                nc.sync.dma_start(m_raws[b][:, :], mk_v[b])

            def prep(b):
                """mf100 = 100*mask (+ npos accum in the same op) and
                t'' = t + 100*mask.  b==0 on DVE (idle ramp); b>0: mf100 on
                Act (Copy w/ scale+accum), t2 on Pool."""
                mf100s[b] = pM.tile([128, PX], f32, tag="mf100", name="mf100", bufs=4)
                t2s[b] = pM.tile([128, NJ, PX], f32, tag="t2", name="t2", bufs=2)
                npcol = accs[:, 33 + b : 34 + b]
                if b == 0:
                    with at(m_end[0] + SEM):
                        nc.vector.tensor_scalar(
                            out=mf100s[b][:, :], in0=m_raws[b][:, :],
                            scalar1=100.0, scalar2=0.0,
                            op0=Alu.mult, op1=Alu.add, accum_out=npcol)
                else:
                    with at(m_end[b] + SEM):
                        nc.scalar.activation(
                            mf100s[b][:, :], m_raws[b][:, :], Copy,
                            bias=0.0, scale=100.0, accum_out=npcol)
                t_raw_v = t_raws[b][:, :].rearrange("p (px j) -> p j px", j=NJ)
                eng = nc.vector if b == 0 else nc.gpsimd
                # b>0: t2 runs on Pool AFTER s16 of the preceding quantum
                # (whose ts is L+2800) so a waiting t2 never blocks it
                t2_ts = m_end[b] + SEM + (200 if b == 0 else 600)
                with at(t2_ts):
                    eng.tensor_tensor(
                        out=t2s[b][:, :, :], in0=t_raw_v,
                        in1=mf100s[b][:, :].unsqueeze(1).broadcast_to((128, NJ, PX)),
                        op=Alu.add)

            pair_s4 = [None] * 10
            pair_lse = [None] * 10

            def emit_ln(p, base, npages):
                """ln for s4-pair tile p. Emission position sets Act-queue
                priority: keep it AFTER any exp it could head-block."""
                sh = (128, npages, PX) if npages > 1 else (128, PX)
                s4v = pair_s4[p][:, :, :] if npages > 1 else pair_s4[p][:, :]
                pair_lse[p] = pS.tile(list(sh), f32, tag="lse1", name="lse1", bufs=4)
                lsev = pair_lse[p][:, :, :] if npages > 1 else pair_lse[p][:, :]
                with at(base):
                    nc.scalar.activation(lsev, s4v, Ln)

            def emit_ttr(p, base, npages):
                b = min((2 * p) // NJ, BPC - 1)
                sh = (128, npages, PX) if npages > 1 else (128, PX)
                lsev = pair_lse[p][:, :, :] if npages > 1 else pair_lse[p][:, :]
                lscr = pS.tile(list(sh), bf16, tag="lscr", name="lscr", bufs=3)
                lscrv = lscr[:, :, :] if npages > 1 else lscr[:, :]
                mfv = (mf100s[b][:, :].unsqueeze(1).broadcast_to((128, npages, PX))
                       if npages > 1 else mf100s[b][:, :])
                with at(base):
                    nc.vector._custom_dve(
                        ttr_op, out=lscrv, in0=lsev, in1=mfv,
                        s0=0.0, s1=0.01,
                        accum_out=accs[:, 16 + p : 17 + p])

            def emit_pair_ln_ttr(p, base, npages):
                emit_ln(p, base, npages)
                emit_ttr(p, base + 800, npages)

            LAST = BPC * NJ - 1

            dma_tm(0)
            prep(0)

            for u in range(BPC * NJ):
                b, j = divmod(u, NJ)

                if u < LAST:
                    L = pL.tile([128, BINS, PX], f32, tag="L", bufs=6)
                    nc.sync.dma_start(L[:, :, :], x_v[b, :, 8 * j : 8 * j + 8, :])
                else:
                    La = pL.tile([128, 6, PX], f32, tag="La", bufs=1)
                    Lb = pL.tile([128, 2, PX], f32, tag="Lb", bufs=1)
                    nc.sync.dma_start(La[:, :, :], x_v[b, :, 8 * j : 8 * j + 6, :])
                    nc.sync.dma_start(Lb[:, :, :], x_v[b, :, 8 * j + 6 : 8 * j + 8, :])
                if j == 2 and b < BPC - 1:
                    dma_tm(b + 1)
                    prep(b + 1)

                t2v = t2s[b][:, j, :].unsqueeze(1)
                if u < LAST:
                    # DVE: hat
                    hat_scr = pE.tile([128, BINS, PX], bf16, tag="hat", bufs=2)
                    with at(L_end[u] + SEM):
                        nc.vector._custom_dve(
                            hat_op, out=hat_scr[:, :, :],
                            in0=t2v.broadcast_to((128, BINS, PX)),
                            in1=L[:, :, :], s0=100.0, s1=1.0,
                            accum_out=accs[:, u : u + 1])
                    # Act: exp
                    E = pE.tile([128, BINS, PX], bf16, tag="E", bufs=3)
                    with at(L_end[u] + SEM):
                        nc.scalar.activation(E[:, :, :], L[:, :, :], Exp)
                    # Pool: s16; DVE: s8, s4
                    s16s[u] = pS.tile([128, 4, PX], bf16, tag="s16", name="s16", bufs=4)
                    with at(L_end[u] + SEM + 1850):
                        nc.gpsimd.tensor_tensor(
                            out=s16s[u][:, :, :], in0=E[:, 0::2, :],
                            in1=E[:, 1::2, :], op=Alu.add)
                    s8s[u] = pS.tile([128, 2, PX], bf16, tag="s8", name="s8", bufs=3)
                    with at(L_end[u] + SEM + 3950):
                        nc.vector.tensor_tensor(
                            out=s8s[u][:, :, :], in0=s16s[u][:, 0::2, :],
                            in1=s16s[u][:, 1::2, :], op=Alu.add)
                    if u < 14:
                        p = u // 2
                        if u % 2 == 0:
                            pair_s4[p] = pS.tile([128, 2, PX], bf16, tag="s4j", name="s4j", bufs=4)
                        with at(L_end[u] + SEM + 4150):
                            nc.vector.tensor_tensor(
                                out=pair_s4[p][:, u % 2, :], in0=s8s[u][:, 0, :],
                                in1=s8s[u][:, 1, :], op=Alu.add)

                    else:  # u == 14: singleton
                        p = 7
                        pair_s4[p] = pS.tile([128, PX], bf16, tag="s4s", name="s4s", bufs=4)
                        with at(L_end[u] + SEM + 4150):
                            nc.vector.tensor_tensor(
                                out=pair_s4[p][:, :], in0=s8s[u][:, 0, :],
                                in1=s8s[u][:, 1, :], op=Alu.add)
                else:
                    # final tile: two DMA halves; hat halves on DVE; the
                    # sumexp tree runs fully on DVE (Pool is slower and the
                    # chain is latency-critical here)
                    TA = L_end[15] + SEM
                    TB = L_end[16] + SEM
                    hat_a = pE.tile([128, 6, PX], bf16, tag="hata", bufs=1)
                    with at(TA):
                        nc.vector._custom_dve(
                            hat_op, out=hat_a[:, :, :],
                            in0=t2v.broadcast_to((128, 6, PX)),
                            in1=La[:, :, :], s0=100.0, s1=1.0,
                            accum_out=accs[:, 15:16])
                    Ea = pE.tile([128, 6, PX], bf16, tag="Ea", bufs=1)
                    with at(TA):
                        nc.scalar.activation(Ea[:, :, :], La[:, :, :], Exp)
                    hat_b = pE.tile([128, 2, PX], bf16, tag="hatb", bufs=1)
                    with at(TB):
                        nc.vector._custom_dve(
                            hat_op, out=hat_b[:, :, :],
                            in0=t2v.broadcast_to((128, 2, PX)),
                            in1=Lb[:, :, :], s0=106.0, s1=1.0,
                            accum_out=accs[:, 32:33])
                    Eb = pE.tile([128, 2, PX], bf16, tag="Eb", bufs=1)
                    with at(TB):
                        nc.scalar.activation(Eb[:, :, :], Lb[:, :, :], Exp)
                    # 6-page partial sum (pairs -> 3 -> 1), ready before Eb lands
                    sA = pS.tile([128, 3, PX], bf16, tag="sA", bufs=1)
                    with at(TA + 1500):
                        nc.vector.tensor_tensor(
                            out=sA[:, :, :], in0=Ea[:, 0::2, :],
                            in1=Ea[:, 1::2, :], op=Alu.add)
                    qA = pS.tile([128, PX], bf16, tag="q15", bufs=1)
                    with at(TA + 1800):
                        nc.vector.tensor_tensor(
                            out=qA[:, :], in0=sA[:, 0, :], in1=sA[:, 1, :], op=Alu.add)
                    qB = pS.tile([128, PX], bf16, tag="qB", bufs=1)
                    with at(TA + 2000):
                        nc.vector.tensor_tensor(
                            out=qB[:, :], in0=qA[:, :], in1=sA[:, 2, :], op=Alu.add)
                    # last 2 pages join after Eb
                    sB = pS.tile([128, PX], bf16, tag="sB", bufs=1)
                    with at(TB + 1150):
                        nc.vector.tensor_tensor(
                            out=sB[:, :], in0=Eb[:, 0, :], in1=Eb[:, 1, :], op=Alu.add)
                    pair_s4[8] = pS.tile([128, PX], bf16, tag="s4s", name="s4s", bufs=4)
                    with at(TB + 1350):
                        nc.vector.tensor_tensor(
                            out=pair_s4[8][:, :], in0=qB[:, :], in1=sB[:, :], op=Alu.add)
                    # deferred lns/ttrs for the late pairs, in queue order
                    # AFTER the final exps: p6 (u12,u13), p7 (u14), p8 (u15)
                    emit_ln(6, 45100, 2)
                    emit_ln(7, 45600, 1)
                    emit_ttr(6, 45900, 2)
                    emit_ln(8, 46100, 1)
                    emit_ttr(7, 46400, 1)
                    emit_ttr(8, 46900, 1)

                # pairs p0..p5: emit two iterations after completion so their
                # ln sits BEHIND this iteration's exp in the Act queue
                if u % 2 == 0 and 4 <= u <= 14:
                    pd = (u - 4) // 2
                    emit_pair_ln_ttr(pd, L_end[u - 3] + SEM + 4550, 2)

            with at(44500):
                nc.sync.dma_start(acc_out[:, :], accs[:, :])

    nc.finalize()
    return nc


def _get_nc():
    with _lock:
        if "nc" not in _cache:
            _cache["nc"] = _build_nc()
        return _cache["nc"]


def kernel(reg_logits: np.ndarray, targets: np.ndarray, pos_mask: np.ndarray) -> np.ndarray:
    from concourse.bass_utils import run_bass_kernel_spmd

    nc = _get_nc()

    reg_logits = np.ascontiguousarray(reg_logits, dtype=np.float32).reshape(B, C, HW)
    targets = np.ascontiguousarray(targets, dtype=np.float32)
    mask_u8 = np.ascontiguousarray(pos_mask).astype(np.uint8)

    in_maps = []
    for c in range(NCORES):
        b0 = c * BPC
        in_maps.append(
            {
                "x": reg_logits[b0 : b0 + BPC],
                "tg": targets[b0 : b0 + BPC],
                "mk": mask_u8[b0 : b0 + BPC],
            }
        )

    res = run_bass_kernel_spmd(nc, in_maps, core_ids=list(range(NCORES)))

    tot_interp = 0.0
    tot_lse = 0.0
    npos100 = 0.0
    for r in res.results:
        a = r["acc"].astype(np.float64)
        tot_interp += a[:, 0:16].sum() + a[:, 32:33].sum()
        tot_lse += a[:, 16:25].sum()
        npos100 += a[:, 33:37].sum()

    npos = npos100 / 100.0
    total = tot_lse - tot_interp
    loss = total / (max(npos, 1.0) * 4.0) if npos > 0 else 0.0
    return np.float32(loss)


if __name__ == "__main__":
    rng = np.random.default_rng(0)
    rl = rng.standard_normal((B, C, H, W), dtype=np.float32)
    tg = (rng.random((B, HW, NJ), dtype=np.float32) * (BINS - 1)).astype(np.float32)
    pm = rng.integers(0, 2, size=(B, HW)).astype(bool)
    print(kernel(reg_logits=rl, targets=tg, pos_mask=pm))
